# revision 1
# baseline (speedup 1.0000x reference)
"""Multi-head attention kernel for Trainium2 (8 NeuronCores, data-parallel over batch).

Reference computation (per batch b of 8):
    x:  [1024, 768]  (tokens x channels, n = 32*32)
    qkv = x @ qkv_w.T                    -> [1024, 2304]
    q, k, v per head (12 heads, dh=64)
    S = q @ k.T * dh**-0.5; P = softmax(S); O = P @ v
    out = concat_heads(O) @ proj_w.T + proj_b

Each core processes one batch element independently (no collectives).

On-chip layouts (bf16 compute, fp32 PSUM accumulation):
    x_all     [128c, 6ct, 1024t]          (x^T: c on partitions, tiled by ct)
    wq_all    [128c, 6ct, 2304o]          (qkv_w^T)
    wp_all    [128c, 6ct, 768o]           (proj_w^T)
    qkT[i]    [128o, 1024t]  i=0..11      (q^T tiles 0-5, k^T tiles 6-11)
    V[tt]     [128t, 12h, 65]             (v natural + ones column per head)
    E[h]      [128j, 8jt, 1024i]          (exp(S^T) per head, bf16)
    OT[g]     [128c, 1024t]  g=0..5       (attention out transposed, head pairs)

All operand transposes ride the tensor engine (transpose-mode matmul vs a
bf16 identity, fine-grained per 128x128 block) so compute can start as soon
as x and the first weight row-tiles are resident - no DMA-transpose xbar, no
DRAM bounce.

Attention per head uses the transposed-score trick (no max subtraction -
scores are O(1) for this distribution and exp runs in fp32):
    S^T[j,i] = sum_d k^T[d,j] q^T[d,i]       (matmul, K=dh=64, head pairs
                                              row-packed on the PE array)
    E^T = exp(S^T * scale)                    (ACT, PSUM->SBUF, bf16)
    [O^T | colsum] = [V|1]^T @ E^T            (matmul, K=128 over j tiles)
    O^T /= colsum                             (reciprocal + step-0-AP DMA
                                              broadcast + DVE multiply)
"""

import numpy as np

import concourse.bass as bass
import concourse.mybir as mybir
import concourse.tile as tile
from concourse import bacc
from concourse.masks import make_identity

# Problem constants (hardcoded per contract)
B = 8
N = 1024          # tokens per batch (32*32)
C = 768           # channels
H = 12            # heads
DH = 64           # head dim
O3 = 3 * C        # 2304
SCALE = DH ** -0.5
NCORES = 8

F32 = mybir.dt.float32
BF16 = mybir.dt.bfloat16

CT = C // 128     # 6 c-tiles
TT = N // 128     # 8 token tiles
IC = N // 512     # 2 i-chunks of 512
JT = N // 128     # 8 j-tiles


def _build_nc(dbg=False, repeat=1):
    nc = bacc.Bacc("TRN2", target_bir_lowering=False, debug=False, num_devices=NCORES)

    x_d = nc.dram_tensor("x", [N, C], F32, kind="ExternalInput").ap()
    qkvw_d = nc.dram_tensor("qkv_w", [O3, C], F32, kind="ExternalInput").ap()
    projw_d = nc.dram_tensor("proj_w", [C, C], F32, kind="ExternalInput").ap()
    projb_d = nc.dram_tensor("proj_b", [C], F32, kind="ExternalInput").ap()
    out_d = nc.dram_tensor("out", [N, C], F32, kind="ExternalOutput").ap()

    with tile.TileContext(nc) as tc:
        _emit(nc, tc, x_d, qkvw_d, projw_d, projb_d, out_d, dbg=dbg, repeat=repeat)
    nc.compile()
    return nc


def _emit(nc, tc, x_d, qkvw_d, projw_d, projb_d, out_d, dbg=False, repeat=1):
    from contextlib import ExitStack

    with ExitStack() as ctx:
        # ---------------- pools ----------------
        sb = lambda name, bufs: ctx.enter_context(tc.tile_pool(name=name, bufs=bufs))
        ps = lambda name, bufs: ctx.enter_context(
            tc.tile_pool(name=name, bufs=bufs, space="PSUM")
        )

        stage_pool = sb("stage", 3)      # fp32 load staging [128, 768]
        bfst_pool = sb("bfst", 3)        # bf16 cast staging [128, 768]
        big_pool = sb("big", 1)          # x_all / wq_all / wp_all / identity
        qkT_pool = sb("qkT", 12)
        v_pool = sb("vbf", TT)
        e_pool = sb("ebf", 2)
        ot_sb_pool = sb("otsb", CT)
        pjp_pool = sb("pjpart", TT)
        rec_pool = sb("rec", 5)
        bias_pool = sb("bias", 1)
        out_pool = sb("outsb", 4)

        qkv_ps = ps("qkvps", 2)          # 1 bank each: transposes + QKV + proj
        sps_ps = ps("sps", 2)            # 2 banks each: S^T units
        ot_ps = ps("otps", 2)            # 1 bank each: O^T + colsum

        # ---------------- persistent tiles ----------------
        ident = big_pool.tile([128, 128], BF16, tag="ident")
        make_identity(nc, ident)

        x_all = big_pool.tile([128, CT, N], BF16, tag="x_all")
        wq_all = big_pool.tile([128, CT, O3], BF16, tag="wq_all")
        wp_all = big_pool.tile([128, CT, C], BF16, tag="wp_all")

        qkT = [
            qkT_pool.tile([128, N], BF16, tag="qkT", name=f"qkT_{i}") for i in range(12)
        ]
        Vt = [
            v_pool.tile([128, H, DH + 1], BF16, tag="vbf", name=f"V_{i}")
            for i in range(TT)
        ]
        OT = [
            ot_sb_pool.tile([128, N], BF16, tag="otsb", name=f"OT_{i}")
            for i in range(CT)
        ]

        # bias broadcast to all partitions (fp32)
        bias_row = bias_pool.tile([1, C], F32, tag="biasrow")
        nc.gpsimd.dma_start(out=bias_row, in_=projb_d[None, :])
        bias_bc = bias_pool.tile([128, C], F32, tag="biasbc")
        nc.gpsimd.partition_broadcast(bias_bc, bias_row)

        # ---------------- load + cast + PE-transpose one row-tile ----------------
        # src row-tile rt of a [rows, 768] fp32 matrix -> dst_all[:, :, rt*128+...]
        def load_rt(src, rt, dst_all, qi):
            stg = stage_pool.tile([128, C], F32, tag="stage", name=f"stg_{rt}")
            eng = nc.scalar if qi % 2 else nc.sync
            eng.dma_start(out=stg, in_=src[rt * 128:(rt + 1) * 128, :])
            stgb = bfst_pool.tile([128, C], BF16, tag="bfst", name=f"stgb_{rt}")
            nc.vector.tensor_copy(stgb, stg)
            tp = qkv_ps.tile([128, CT, 128], BF16, tag="qkvps", name="tp")
            for ct in range(CT):
                nc.tensor.transpose(tp[:, ct, :], stgb[:, ct * 128:(ct + 1) * 128], ident)
            nc.vector.tensor_copy(dst_all[:, :, rt * 128:(rt + 1) * 128], tp)

        # ---------------- compute phases ----------------
        def qk_pair(g):
            # q rows o in [g*128, +128); k rows o in [768 + g*128, +128)
            for obase, dst in ((g * 128, qkT[g]), (C + g * 128, qkT[6 + g])):
                for ic in range(IC):
                    acc = qkv_ps.tile([128, 512], F32, tag="qkvps", name="qk_acc")
                    for ct in range(CT):
                        nc.tensor.matmul(
                            acc,
                            lhsT=wq_all[:, ct, obase:obase + 128],
                            rhs=x_all[:, ct, ic * 512:(ic + 1) * 512],
                            start=(ct == 0),
                            stop=(ct == CT - 1),
                        )
                    nc.vector.tensor_copy(dst[:, ic * 512:(ic + 1) * 512], acc)

        def v_chunk(oc):
            # v rows o in [1536 + oc*384, +384) -> heads 6*oc .. 6*oc+5
            for tt in range(TT):
                acc = qkv_ps.tile([128, 384], F32, tag="qkvps", name="v_acc")
                for ct in range(CT):
                    nc.tensor.matmul(
                        acc,
                        lhsT=x_all[:, ct, tt * 128:(tt + 1) * 128],
                        rhs=wq_all[:, ct, 2 * C + oc * 384:2 * C + (oc + 1) * 384],
                        start=(ct == 0),
                        stop=(ct == CT - 1),
                    )
                if oc == 0:
                    nc.vector.memset(Vt[tt][:, :, DH:DH + 1], 1.0)
                nc.vector.tensor_copy(
                    Vt[tt][:, 6 * oc:6 * (oc + 1), 0:DH],
                    acc.rearrange("p (h d) -> p h d", d=DH),
                )

        def attn_pair(g):
            h0, h1 = 2 * g, 2 * g + 1
            E = {}
            for h in (h0, h1):
                E[h] = e_pool.tile([128, JT, N], BF16, tag="ebf", name=f"E_{h}")
            # S^T + exp: unit = 2 j-tiles for one head; heads interleaved for
            # PE row-packing (h0 rows 0-63, h1 rows 64-127 of the qkT tiles).
            for ic in range(IC):
                for u in range(JT // 2):
                    un = {}
                    for h in (h0, h1):
                        un[h] = sps_ps.tile([128, 2, 512], F32, tag="sps", name="sT")
                    for q in range(2):
                        jt = 2 * u + q
                        for h in (h0, h1):
                            hoff = (h % 2) * DH
                            nc.tensor.matmul(
                                un[h][:, q, :],
                                lhsT=qkT[6 + g][hoff:hoff + DH, jt * 128:(jt + 1) * 128],
                                rhs=qkT[g][hoff:hoff + DH, ic * 512:(ic + 1) * 512],
                                start=True,
                                stop=True,
                            )
                    for h in (h0, h1):
                        nc.scalar.activation(
                            E[h][:, 2 * u:2 * u + 2, ic * 512:(ic + 1) * 512],
                            un[h],
                            mybir.ActivationFunctionType.Exp,
                            scale=SCALE,
                        )
            # O^T (+colsum via ones column), then normalize. ic-outer so both
            # heads' first OT halves land early and the final projection's
            # first token tiles can start during the last pair's second half.
            for ic in range(IC):
                for h in (h0, h1):
                    hoff = (h % 2) * DH
                    acc = ot_ps.tile([65, 512], F32, tag="otps", name="ot_acc")
                    for jt in range(JT):
                        nc.tensor.matmul(
                            acc,
                            lhsT=Vt[jt][:, h, :],
                            rhs=E[h][:, jt, ic * 512:(ic + 1) * 512],
                            start=(jt == 0),
                            stop=(jt == JT - 1),
                        )
                    rec = rec_pool.tile([65, 512], F32, tag="rec", name="rec")
                    nc.vector.reciprocal(rec[64:65, :], acc[64:65, :])
                    # broadcast the reciprocal row across 64 partitions via a
                    # step-0 free-dim DMA (gpsimd.partition_broadcast ignores
                    # the AP partition offset on HW).
                    rbc = rec_pool.tile([64, 512], F32, tag="rbc", name="rbc")
                    row = rec[64:65, :]
                    src = bass.AP(row.tensor, row.offset, [row.ap[0], [0, 64], [1, 512]])
                    nc.gpsimd.dma_start(out=rbc, in_=src)
                    # DVE handles 64-aligned partition shift: odd head writes
                    # partitions 64-127 of OT while reading acc at 0-63.
                    dst = OT[g][hoff:hoff + 64, ic * 512:(ic + 1) * 512]
                    nc.vector.tensor_mul(dst, acc[0:64, :], rbc)
            if dbg and g == 0:
                for h, Eh in E.items():
                    d = nc.dram_tensor(
                        f"dbg_Etap{h}", list(Eh.shape), Eh.dtype, kind="ExternalOutput"
                    ).ap()
                    nc.gpsimd.dma_start(out=d, in_=Eh)

        # proj is split so the g=0..3 partial sums (head pairs 0-3) overlap
        # the ACT-bound tail of attention pairs 4-5; bias is folded into the
        # bf16 partial, the last two pairs accumulate in PSUM.
        pj_partial = [
            pjp_pool.tile([128, C], BF16, tag="pjpart", name=f"pjp_{i}")
            for i in range(TT)
        ]

        def proj_pass1():
            for tt in range(TT):
                for oc in range(2):
                    acc = qkv_ps.tile([128, 384], F32, tag="qkvps", name="pj_acc")
                    for g in range(4):
                        nc.tensor.matmul(
                            acc,
                            lhsT=OT[g][:, tt * 128:(tt + 1) * 128],
                            rhs=wp_all[:, g, oc * 384:(oc + 1) * 384],
                            start=(g == 0),
                            stop=(g == 3),
                        )
                    nc.vector.tensor_add(
                        pj_partial[tt][:, oc * 384:(oc + 1) * 384],
                        acc,
                        bias_bc[:, oc * 384:(oc + 1) * 384],
                    )

        def proj_pass2():
            for tt in range(TT):
                osb = out_pool.tile([128, C], F32, tag="outsb", name="osb")
                for oc in range(2):
                    acc = qkv_ps.tile([128, 384], F32, tag="qkvps", name="pj_acc")
                    for g in (4, 5):
                        nc.tensor.matmul(
                            acc,
                            lhsT=OT[g][:, tt * 128:(tt + 1) * 128],
                            rhs=wp_all[:, g, oc * 384:(oc + 1) * 384],
                            start=(g == 4),
                            stop=(g == 5),
                        )
                    nc.vector.tensor_add(
                        osb[:, oc * 384:(oc + 1) * 384],
                        acc,
                        pj_partial[tt][:, oc * 384:(oc + 1) * 384],
                    )
                # HWDGE queue (ACT is idle by now); SWDGE adds ~1.5us latency
                nc.scalar.dma_start(
                    out=out_d[tt * 128:(tt + 1) * 128, :], in_=osb
                )

        # ---------------- emission schedule ----------------
        # x first (QKV needs all of x^T), then weight row-tiles in the order
        # the pair-pipelined compute consumes them. Attention (ACT-heavy)
        # overlaps QKV matmuls of later pairs.
        qi = 0

        def load_w(rt):
            nonlocal qi
            load_rt(qkvw_d, rt, wq_all, qi); qi += 1

        for _ in range(repeat):
            for rt in range(TT):
                load_rt(x_d, rt, x_all, qi); qi += 1
            # start the exp stream (near-critical ACT resource) as early as
            # possible: attention pair 0 right after its own q/k tiles + v
            load_w(0); load_w(6); load_w(12); load_w(13); load_w(14)
            qk_pair(0)
            v_chunk(0)
            attn_pair(0)
            load_w(1); load_w(7)
            qk_pair(1)
            attn_pair(1)
            load_w(2); load_w(8)
            qk_pair(2)
            attn_pair(2)
            load_w(3); load_w(9)
            qk_pair(3)
            load_w(15); load_w(16); load_w(17)
            v_chunk(1)
            attn_pair(3)
            load_w(4); load_w(10)
            qk_pair(4)
            attn_pair(4)
            load_w(5); load_w(11)
            qk_pair(5)
            for rt in range(CT):
                load_rt(projw_d, rt, wp_all, qi); qi += 1
            proj_pass1()
            attn_pair(5)
            proj_pass2()

        if dbg:
            taps = {
                "dbg_xall": x_all,
                "dbg_wqall": wq_all,
                "dbg_qkT0": qkT[0],
                "dbg_qkT6": qkT[6],
                "dbg_V0": Vt[0],
                "dbg_OT0": OT[0],
                "dbg_bias": bias_bc,
            }
            for name, t in taps.items():
                d = nc.dram_tensor(name, list(t.shape), t.dtype, kind="ExternalOutput").ap()
                nc.gpsimd.dma_start(out=d, in_=t)


_NC_CACHE = None


def _get_nc():
    global _NC_CACHE
    if _NC_CACHE is None:
        _NC_CACHE = _build_nc()
    return _NC_CACHE


def kernel(x, qkv_w, proj_w, proj_b, _trace=False):
    from concourse.bass_utils import run_bass_kernel_spmd

    x = np.ascontiguousarray(np.asarray(x, dtype=np.float32))
    qkv_w = np.ascontiguousarray(np.asarray(qkv_w, dtype=np.float32))
    proj_w = np.ascontiguousarray(np.asarray(proj_w, dtype=np.float32))
    proj_b = np.ascontiguousarray(np.asarray(proj_b, dtype=np.float32))

    b, hh, ww, c = x.shape
    assert (b, hh, ww, c) == (B, 32, 32, C)
    xf = x.reshape(B, N, C)

    nc = _get_nc()
    in_maps = [
        {"x": xf[i], "qkv_w": qkv_w, "proj_w": proj_w, "proj_b": proj_b}
        for i in range(NCORES)
    ]
    res = run_bass_kernel_spmd(nc, in_maps, core_ids=list(range(NCORES)), trace=_trace)
    out = np.stack([r["out"] for r in res.results], axis=0).reshape(B, 32, 32, C)
    if _trace:
        kernel._last_results = res
    return out



# revision 51
# speedup vs baseline: 1.3470x; 1.3470x over previous
"""Multi-head attention kernel for Trainium2 (8 NeuronCores, data-parallel over batch).

Reference computation (per batch b of 8):
    x:  [1024, 768]  (tokens x channels, n = 32*32)
    qkv = x @ qkv_w.T                    -> [1024, 2304]
    q, k, v per head (12 heads, dh=64)
    S = q @ k.T * dh**-0.5; P = softmax(S); O = P @ v
    out = concat_heads(O) @ proj_w.T + proj_b

Each core processes one batch element independently (no collectives).

On-chip layouts (bf16 compute, fp32 PSUM accumulation):
    x_all     [128c, 6ct, 1024t]          (x^T: c on partitions, tiled by ct)
    wq_all    [128c, 6ct, 2304o]          (qkv_w^T)
    wp_all    [128c, 6ct, 768o]           (proj_w^T)
    qkT[i]    [128o, 1024t]  i=0..11      (q^T tiles 0-5, k^T tiles 6-11)
    V[tt]     [128t, 12h, 65]             (v natural + ones column per head)
    E[h]      [128j, 8jt, 1024i]          (exp(S^T) per head, bf16)
    OT[g]     [128c, 1024t]  g=0..5       (attention out transposed, head pairs)

All operand transposes ride the tensor engine (transpose-mode matmul vs a
bf16 identity, fine-grained per 128x128 block) so compute can start as soon
as x and the first weight row-tiles are resident - no DMA-transpose xbar, no
DRAM bounce.

Attention per head uses the transposed-score trick (no max subtraction -
scores are O(1) for this distribution and exp runs in fp32):
    S^T[j,i] = sum_d k^T[d,j] q^T[d,i]       (matmul, K=dh=64, head pairs
                                              row-packed on the PE array)
    E^T = exp(S^T * scale)                    (ACT, PSUM->SBUF, bf16)
    [O^T | colsum] = [V|1]^T @ E^T            (matmul, K=128 over j tiles)
    O^T /= colsum                             (reciprocal + step-0-AP DMA
                                              broadcast + DVE multiply)
"""

import numpy as np

import concourse.bass as bass
import concourse.mybir as mybir
import concourse.tile as tile
from concourse import bacc
from concourse.masks import make_identity

# Problem constants (hardcoded per contract)
B = 8
N = 1024          # tokens per batch (32*32)
C = 768           # channels
H = 12            # heads
DH = 64           # head dim
O3 = 3 * C        # 2304
SCALE = DH ** -0.5
NCORES = 8

F32 = mybir.dt.float32
BF16 = mybir.dt.bfloat16

CT = C // 128     # 6 c-tiles
TT = N // 128     # 8 token tiles
IC = N // 512     # 2 i-chunks of 512
JT = N // 128     # 8 j-tiles


def _build_nc(dbg=False, repeat=1, phases=None):
    nc = bacc.Bacc("TRN2", target_bir_lowering=False, debug=False, num_devices=NCORES)

    x_d = nc.dram_tensor("x", [N, C], F32, kind="ExternalInput").ap()
    qkvw_d = nc.dram_tensor("qkv_w", [O3, C], F32, kind="ExternalInput").ap()
    projw_d = nc.dram_tensor("proj_w", [C, C], F32, kind="ExternalInput").ap()
    projb_d = nc.dram_tensor("proj_b", [C], F32, kind="ExternalInput").ap()
    out_d = nc.dram_tensor("out", [N, C], F32, kind="ExternalOutput").ap()

    with tile.TileContext(nc) as tc:
        _emit(
            nc, tc, x_d, qkvw_d, projw_d, projb_d, out_d,
            dbg=dbg, repeat=repeat, phases=phases,
        )
    nc.compile()
    return nc


def _emit(nc, tc, x_d, qkvw_d, projw_d, projb_d, out_d, dbg=False, repeat=1,
          phases=None):
    # phases: ablation support for timing experiments. None = full kernel.
    # Cumulative: "qkv" ⊂ "scores" ⊂ "av" ⊂ "proj" (full).
    P = phases if phases is not None else {"qkv", "scores", "av", "proj"}
    from contextlib import ExitStack

    with ExitStack() as ctx:
        # ---------------- pools ----------------
        sb = lambda name, bufs: ctx.enter_context(tc.tile_pool(name=name, bufs=bufs))
        ps = lambda name, bufs: ctx.enter_context(
            tc.tile_pool(name=name, bufs=bufs, space="PSUM")
        )

        stage_pool = sb("stage", 2)      # fp32 load staging [128, 768]
        bfst_pool = sb("bfst", 2)        # bf16 cast staging [128, 768]
        big_pool = sb("big", 1)          # x_all / wq_all / wp_all / identity
        qkT_pool = sb("qkT", 12)
        v_pool = sb("vbf", TT)
        e_pool = sb("ebf", 4)   # 2 pairs of E tiles in flight: pair g+1's
                                # exp stream must not stall on pair g's AV
        ot_sb_pool = sb("otsb", CT)
        pjp_pool = sb("pjpart", TT)
        rec_pool = sb("rec", 4)
        onorm_pool = sb("onorm", 5)
        bias_pool = sb("bias", 1)
        out_pool = sb("outsb", 2)

        qkv_ps = ps("qkvps", 2)          # 1 bank each: transposes + QKV + proj
        sps_ps = ps("sps", 2)            # 2 banks each: S^T units
        ot_ps = ps("otps", 2)            # 1 bank each: O^T + colsum

        # Timing-variant probe sink: neuronx-cc dead-code-eliminates compute
        # whose results feed no output, so ablation variants anchor their
        # last phase with thin partition-0/64 probe DMAs into this scratch
        # output. Unused (None) for the full kernel.
        sink = None
        sink_col = [0]
        if P != {"qkv", "scores", "av", "proj"}:
            sink = nc.dram_tensor(
                "dce_sink", [64, 131072], BF16, kind="ExternalOutput"
            ).ap()

        def probe(ap_like):
            n = 1
            for d in ap_like.shape[1:]:
                n *= d
            rows = ap_like.shape[0]
            if len(ap_like.shape) == 3:
                ap_like = ap_like.rearrange("p a b -> p (a b)")
            col = sink_col[0]
            nc.sync.dma_start(out=sink[0:rows, col:col + n], in_=ap_like)
            sink_col[0] = col + n

        # ---------------- persistent tiles ----------------
        ident = big_pool.tile([128, 128], BF16, tag="ident")
        make_identity(nc, ident)

        x_all = big_pool.tile([128, CT, N], BF16, tag="x_all")
        wq_all = big_pool.tile([128, CT, O3], BF16, tag="wq_all")
        wp_all = big_pool.tile([128, CT, C], BF16, tag="wp_all")

        qkT = [
            qkT_pool.tile([128, N], BF16, tag="qkT", name=f"qkT_{i}") for i in range(12)
        ]
        Vt = [
            v_pool.tile([128, H, DH + 1], BF16, tag="vbf", name=f"V_{i}")
            for i in range(TT)
        ]
        OT = [
            ot_sb_pool.tile([128, N], BF16, tag="otsb", name=f"OT_{i}")
            for i in range(CT)
        ]

        # bias broadcast to all partitions (fp32)
        bias_row = bias_pool.tile([1, C], F32, tag="biasrow")
        nc.gpsimd.dma_start(out=bias_row, in_=projb_d[None, :])
        bias_bc = bias_pool.tile([128, C], F32, tag="biasbc")
        nc.gpsimd.partition_broadcast(bias_bc, bias_row)



        # ---------------- load + cast + PE-transpose one row-tile ----------------
        # src row-tile rt of a [rows, 768] fp32 matrix -> dst_all[:, :, rt*128+...]
        # x loads (early, ACT idle) alternate the SP/ACT HWDGE queues for
        # bandwidth; weight loads ride SP only so the ACT queue stays free
        # for the exp stream it serves during the attention phase.
        def load_rt(src, rt, dst_all, qi, sp_only=False):
            stg = stage_pool.tile([128, C], F32, tag="stage", name=f"stg_{rt}")
            eng = nc.scalar if (qi % 2 and not sp_only) else nc.sync
            eng.dma_start(out=stg, in_=src[rt * 128:(rt + 1) * 128, :])
            stgb = bfst_pool.tile([128, C], BF16, tag="bfst", name=f"stgb_{rt}")
            nc.vector.tensor_copy(stgb, stg)
            tp = qkv_ps.tile([128, CT, 128], BF16, tag="qkvps", name="tp")
            for ct in range(CT):
                nc.tensor.transpose(tp[:, ct, :], stgb[:, ct * 128:(ct + 1) * 128], ident)
            nc.vector.tensor_copy(dst_all[:, :, rt * 128:(rt + 1) * 128], tp)

        # ---------------- compute phases ----------------
        def qk_pair(g):
            # q rows o in [g*128, +128); k rows o in [768 + g*128, +128)
            for obase, dst in ((g * 128, qkT[g]), (C + g * 128, qkT[6 + g])):
                for ic in range(IC):
                    acc = qkv_ps.tile([128, 512], F32, tag="qkvps", name="qk_acc")
                    # qkv4: timing-only PE-throughput calibration (4x stream)
                    for _ in range(4 if "qkv4" in P else 1):
                        for ct in range(CT):
                            nc.tensor.matmul(
                                acc,
                                lhsT=wq_all[:, ct, obase:obase + 128],
                                rhs=x_all[:, ct, ic * 512:(ic + 1) * 512],
                                start=(ct == 0),
                                stop=(ct == CT - 1),
                            )
                    nc.vector.tensor_copy(dst[:, ic * 512:(ic + 1) * 512], acc)

        def v_chunk(oc):
            # v rows o in [1536 + oc*384, +384) -> heads 6*oc .. 6*oc+5
            for tt in range(TT):
                acc = qkv_ps.tile([128, 384], F32, tag="qkvps", name="v_acc")
                for ct in range(CT):
                    nc.tensor.matmul(
                        acc,
                        lhsT=x_all[:, ct, tt * 128:(tt + 1) * 128],
                        rhs=wq_all[:, ct, 2 * C + oc * 384:2 * C + (oc + 1) * 384],
                        start=(ct == 0),
                        stop=(ct == CT - 1),
                    )
                if oc == 0:
                    nc.vector.memset(Vt[tt][:, :, DH:DH + 1], 1.0)
                nc.vector.tensor_copy(
                    Vt[tt][:, 6 * oc:6 * (oc + 1), 0:DH],
                    acc.rearrange("p (h d) -> p h d", d=DH),
                )

        # E tiles handed from scores_part(g) to av_part(g); two pairs live
        # at once so the exp stream never stalls on AV consumption.
        E_live = {}

        def scores_part(g):
            if "scores" not in P:
                return
            h0, h1 = 2 * g, 2 * g + 1
            E = {}
            for h in (h0, h1):
                E[h] = e_pool.tile([128, JT, N], BF16, tag="ebf", name=f"E_{h}")
            E_live[g] = E
            # S^T + exp: unit = 2 j-tiles for one head; heads interleaved for
            # PE row-packing (h0 rows 0-63, h1 rows 64-127 of the qkT tiles).
            for ic in range(IC):
                for u in range(JT // 2):
                    un = {}
                    for h in (h0, h1):
                        un[h] = sps_ps.tile([128, 2, 512], F32, tag="sps", name="sT")
                    for q in range(2):
                        jt = 2 * u + q
                        for h in (h0, h1):
                            hoff = (h % 2) * DH
                            # sc4: timing-only PE calibration (4x stream)
                            for _ in range(4 if "sc4" in P else 1):
                                nc.tensor.matmul(
                                    un[h][:, q, :],
                                    lhsT=qkT[6 + g][hoff:hoff + DH, jt * 128:(jt + 1) * 128],
                                    rhs=qkT[g][hoff:hoff + DH, ic * 512:(ic + 1) * 512],
                                    start=True,
                                    stop=True,
                                )
                    for h in (h0, h1):
                        reps = 4 if "exp4" in P else 1
                        for _ in range(reps):
                            # exp4: timing-only ACT-throughput calibration -
                            # same dst/src, WAW-chained, 4x the exp stream
                            nc.scalar.activation(
                                E[h][:, 2 * u:2 * u + 2, ic * 512:(ic + 1) * 512],
                                un[h],
                                mybir.ActivationFunctionType.Exp,
                                scale=SCALE,
                            )
            if "av" not in P:
                for h in (h0, h1):
                    probe(E[h][0:1, :, :])

        def av_part(g):
            # O-direct AV: out O[i-part, V|den] per (h, it) with lhsT = E tile
            # (M=128, full PE width) and rhs = [V|1] (N=65). The softmax
            # denominator lands in column 64 PER PARTITION, so normalization
            # is a native [P,1]-broadcast tensor_scalar on DVE - no
            # partition-direction broadcast (which cost ~87us/kernel on HW in
            # the O^T formulation). The normalized [128,128] pair-block is
            # PE-transposed back into the OT layout proj expects.
            if "scores" not in P or "av" not in P:
                return
            h0, h1 = 2 * g, 2 * g + 1
            E = E_live.pop(g)
            # 4-wide it-groups: one PSUM tile [128, 4, 65] holds 4 i-blocks'
            # accumulators (1040B, single bank), so 8 units are in flight
            # across the 2 ot_ps bufs and one reciprocal serves 4 blocks.
            for ig in range(TT // 4):
                its = range(4 * ig, 4 * ig + 4)
                obs = {
                    it: onorm_pool.tile(
                        [128, 128], BF16, tag="onorm", name=f"ob_{it}"
                    )
                    for it in its
                }
                for h in (h0, h1):
                    hoff = (h % 2) * DH
                    acc4 = ot_ps.tile([128, 4, DH + 1], F32, tag="otps",
                                      name="od_acc4")
                    for q, it in enumerate(its):
                        for jt in range(JT):
                            nc.tensor.matmul(
                                acc4[:, q, :],
                                lhsT=E[h][:, jt, it * 128:(it + 1) * 128],
                                rhs=Vt[jt][:, h, :],
                                start=(jt == 0),
                                stop=(jt == JT - 1),
                            )
                    if "nonorm" in P:
                        # timing-only ablation: skip softmax normalization
                        for q, it in enumerate(its):
                            nc.vector.tensor_copy(
                                obs[it][:, hoff:hoff + DH], acc4[:, q, 0:DH]
                            )
                        continue
                    rcp4 = rec_pool.tile([128, 4], F32, tag="rec", name="rcp4")
                    nc.vector.reciprocal(
                        rcp4,
                        acc4[:, :, DH:DH + 1].rearrange("p a b -> p (a b)"),
                    )
                    for q, it in enumerate(its):
                        nc.vector.tensor_scalar_mul(
                            obs[it][:, hoff:hoff + DH],
                            acc4[:, q, 0:DH],
                            rcp4[:, q:q + 1],
                        )
                for it in its:
                    tp = qkv_ps.tile([128, 128], BF16, tag="qkvps", name="ob_tp")
                    nc.tensor.transpose(tp, obs[it], ident)
                    nc.vector.tensor_copy(OT[g][:, it * 128:(it + 1) * 128], tp)
            if dbg and g == 0:
                for h, Eh in E.items():
                    d = nc.dram_tensor(
                        f"dbg_Etap{h}", list(Eh.shape), Eh.dtype, kind="ExternalOutput"
                    ).ap()
                    nc.gpsimd.dma_start(out=d, in_=Eh)

        # proj is split so the g=0..3 partial sums (head pairs 0-3) overlap
        # the ACT-bound tail of attention pairs 4-5; bias is folded into the
        # bf16 partial, the last two pairs accumulate in PSUM.
        pj_partial = [
            pjp_pool.tile([128, C], BF16, tag="pjpart", name=f"pjp_{i}")
            for i in range(TT)
        ]

        def proj_pass1():
            for tt in range(TT):
                for oc in range(2):
                    acc = qkv_ps.tile([128, 384], F32, tag="qkvps", name="pj_acc")
                    for g in range(4):
                        nc.tensor.matmul(
                            acc,
                            lhsT=OT[g][:, tt * 128:(tt + 1) * 128],
                            rhs=wp_all[:, g, oc * 384:(oc + 1) * 384],
                            start=(g == 0),
                            stop=(g == 3),
                        )
                    nc.vector.tensor_add(
                        pj_partial[tt][:, oc * 384:(oc + 1) * 384],
                        acc,
                        bias_bc[:, oc * 384:(oc + 1) * 384],
                    )

        def proj_pass2():
            for tt in range(TT):
                osb = out_pool.tile([128, C], F32, tag="outsb", name="osb")
                for oc in range(2):
                    acc = qkv_ps.tile([128, 384], F32, tag="qkvps", name="pj_acc")
                    for g in (4, 5):
                        nc.tensor.matmul(
                            acc,
                            lhsT=OT[g][:, tt * 128:(tt + 1) * 128],
                            rhs=wp_all[:, g, oc * 384:(oc + 1) * 384],
                            start=(g == 4),
                            stop=(g == 5),
                        )
                    nc.vector.tensor_add(
                        osb[:, oc * 384:(oc + 1) * 384],
                        acc,
                        pj_partial[tt][:, oc * 384:(oc + 1) * 384],
                    )
                # HWDGE queue (ACT is idle by now); SWDGE adds ~1.5us latency
                nc.scalar.dma_start(
                    out=out_d[tt * 128:(tt + 1) * 128, :], in_=osb
                )

        # ---------------- emission schedule ----------------
        # x first (QKV needs all of x^T), then weight row-tiles in the order
        # the pair-pipelined compute consumes them. Attention (ACT-heavy)
        # overlaps QKV matmuls of later pairs.
        qi = 0

        def load_w(rt):
            nonlocal qi
            load_rt(qkvw_d, rt, wq_all, qi, sp_only=True); qi += 1

        if "qkv" not in P:
            def qk_pair(g):  # noqa: F811
                pass
            def v_chunk(oc):  # noqa: F811
                pass
        for _ in range(repeat):
            sink_col[0] = 0
            for rt in range(TT):
                load_rt(x_d, rt, x_all, qi); qi += 1
            # Start the exp stream (near-critical ACT resource) as early as
            # possible: attention pair 0 right after its own q/k tiles + v.
            # scores_part(g+1) is emitted BEFORE av_part(g) so the PE fills
            # the next pair's S^T units during pair g's exp stream, and
            # av_part(g) runs under pair g+1's exp - the exp stream never
            # waits behind AV on the PE queue.
            load_w(0); load_w(6); load_w(12); load_w(13); load_w(14)
            qk_pair(0)
            v_chunk(0)
            scores_part(0)
            load_w(1); load_w(7)
            qk_pair(1)
            scores_part(1)
            av_part(0)
            load_w(2); load_w(8)
            qk_pair(2)
            scores_part(2)
            av_part(1)
            load_w(3); load_w(9)
            qk_pair(3)
            load_w(15); load_w(16); load_w(17)
            v_chunk(1)
            scores_part(3)
            av_part(2)
            load_w(4); load_w(10)
            qk_pair(4)
            scores_part(4)
            av_part(3)
            load_w(5); load_w(11)
            qk_pair(5)
            scores_part(5)
            av_part(4)
            for rt in range(CT):
                load_rt(projw_d, rt, wp_all, qi, sp_only=True); qi += 1
            if "proj" in P:
                proj_pass1()
            av_part(5)
            if "proj" in P:
                proj_pass2()

            if sink is not None:
                if "av" in P:
                    for g in range(CT):
                        probe(OT[g][0:2, :])
                        probe(OT[g][64:66, :])
                elif "scores" in P:
                    pass  # probed per-pair inside attn_pair (E pool-cycles)
                elif "qkv" in P:
                    for i in range(12):
                        probe(qkT[i][0:1, :])
                    for tt in range(TT):
                        probe(Vt[tt][0:1, :, :])
                else:
                    probe(x_all[0:1, :, :])
                    probe(wq_all[0:1, :, :])
                    probe(wp_all[0:1, :, :])

        if dbg:
            taps = {
                "dbg_xall": x_all,
                "dbg_wqall": wq_all,
                "dbg_qkT0": qkT[0],
                "dbg_qkT6": qkT[6],
                "dbg_V0": Vt[0],
                "dbg_OT0": OT[0],
                "dbg_bias": bias_bc,
            }
            for name, t in taps.items():
                d = nc.dram_tensor(name, list(t.shape), t.dtype, kind="ExternalOutput").ap()
                nc.gpsimd.dma_start(out=d, in_=t)


_NC_CACHE = None


def _get_nc():
    global _NC_CACHE
    if _NC_CACHE is None:
        _NC_CACHE = _build_nc()
    return _NC_CACHE


def kernel(x, qkv_w, proj_w, proj_b, _trace=False):
    from concourse.bass_utils import run_bass_kernel_spmd

    x = np.ascontiguousarray(np.asarray(x, dtype=np.float32))
    qkv_w = np.ascontiguousarray(np.asarray(qkv_w, dtype=np.float32))
    proj_w = np.ascontiguousarray(np.asarray(proj_w, dtype=np.float32))
    proj_b = np.ascontiguousarray(np.asarray(proj_b, dtype=np.float32))

    b, hh, ww, c = x.shape
    assert (b, hh, ww, c) == (B, 32, 32, C)
    xf = x.reshape(B, N, C)

    nc = _get_nc()
    in_maps = [
        {"x": xf[i], "qkv_w": qkv_w, "proj_w": proj_w, "proj_b": proj_b}
        for i in range(NCORES)
    ]
    res = run_bass_kernel_spmd(nc, in_maps, core_ids=list(range(NCORES)), trace=_trace)
    out = np.stack([r["out"] for r in res.results], axis=0).reshape(B, 32, 32, C)
    if _trace:
        kernel._last_results = res
    return out

